# revision 58
# baseline (speedup 1.0000x reference)
"""Trainium2 Bass kernel for 16-head MultiHeadAttention (B=2, S=2048, D=1024).

Sharding: 8 cores = 2 (batch) x 4 (head groups of 4 heads).
Each core computes the qkv projection for its 4 heads, attention, and a
partial out-projection (TP over heads); the host sums the 4 partials per
batch element.

On-device schedule (~163us/core; PE floor ~137us, ACT exp floor ~128us):
  - Softmax-exp streams on the ACT engine as 8 passes of 16 PSUM score
    slabs [128keys x 1024q] (last pass split into 512-col halves so its
    PV/out-proj can start early). Scores for slab i+2 overlap exp of i.
  - All other PE work (q/k projections split into 2-chunk quarters,
    v-projection halves, PV units, out-proj halves) lives in a global
    FIFO drained after each slab against the ACT pace (612ns/slab fill
    budget, 100ns carry cap), so the PE never outruns or starves ACT.
  - Pass order (h0,qc0)(h0,qc1)(h1,qc0)(h1,qc1)(h2,qc0)(h3,qc0)(h2,qc1)
    (h3,qc1): h0/h1 share q0/k0 and h2/h3 share q1/k1 projections, which
    spreads projection deadlines; qc0 out-projections run mid-stream.
  - PV in transposed orientation (out [q=128, dk+1]) with the softmax
    denominator via a ones-column in V'; normalize = DVE reciprocal +
    tensor_scalar_mul; at rows -> at^T by DMA transpose mid-stream and
    by PE transpose in the tail (serial-DMA latency would gate the end).
  - Tail out-proj halves alternate DVE bias-add and [1-deep ones x b_out
    bias matmul + ACT copy] so DVE and the (by then idle) ACT engine
    each carry half; their PSUM alternates the pso/pq banks (plus the
    ppv bank, idle after the final PV unit, for the last blocks). Projection
    accumulators alternate those same two banks mid-stream (disjoint
    lifetimes) so consecutive projections don't serialize through one
    bank's write-after-read on the bias-add.
  - The cost model's PE clock ramps over ~3us of activity and drops
    after long idles: a skinny [128,128] warm-up matmul train covers the
    initial DMA window so the first projections run at full speed.
  - Head DMAs are split across the SP and ACT HWDGE queues and ordered
    so the first projections' inputs land first.
"""

import sys
from dataclasses import dataclass

for _p in ("/opt/trn_rl_repo",):
    if _p not in sys.path:
        sys.path.insert(0, _p)

import numpy as np

import concourse.bass as bass  # noqa: E402,F401
import concourse.bacc as bacc  # noqa: E402
import concourse.tile as tile  # noqa: E402
from concourse import mybir  # noqa: E402
from concourse.bass_utils import run_bass_kernel_spmd  # noqa: E402

F32 = mybir.dt.float32
BF16 = mybir.dt.bfloat16
AF = mybir.ActivationFunctionType


@dataclass(frozen=True)
class Cfg:
    S: int = 2048      # sequence length
    DIN: int = 1024    # model dim
    HPC: int = 4       # heads per core
    DK: int = 64       # head dim
    N_CORES: int = 8

    @property
    def DQK(self):
        return self.HPC * self.DK  # 256 per-core q/k/v width

    @property
    def KC(self):
        return self.DIN // 128     # 8 contraction chunks

    @property
    def SB(self):
        return self.S // 128       # 16 sequence blocks


FULL = Cfg()


def build_nc(cfg: Cfg = FULL):
    S, DIN, HPC, DK = cfg.S, cfg.DIN, cfg.HPC, cfg.DK
    DQK, KC, SB = cfg.DQK, cfg.KC, cfg.SB
    QC = 1024                 # q-chunk width for attention passes
    SBH = QC // 128           # 8 s-blocks per pass
    SCALE_INV = 1.0 / float(np.sqrt(DK))
    V65 = DK + 1              # V' width per head (denominator ones col)

    nc = bacc.Bacc("TRN2", target_bir_lowering=False, debug=False,
                   num_devices=cfg.N_CORES)

    # x^T in bf16, delivered as 2-s-block slabs: row k2*128+p holds
    # x[k2*256+j, c*128+p] at col c*256+j (see shard_inputs) so each
    # [128,2048] DMA delivers all 8 c-chunks of two s-blocks contiguously.
    xt_d = nc.dram_tensor("xt", [8 * 128, S], BF16, kind="ExternalInput")
    wqkT_d = nc.dram_tensor("w_qkT", [128, 4 * DIN], BF16,
                            kind="ExternalInput")
    wvT_d = nc.dram_tensor("w_vT", [128, KC * DQK], BF16,
                           kind="ExternalInput")
    woT_d = nc.dram_tensor("w_oT", [128, 2 * DIN], BF16,
                           kind="ExternalInput")
    bqk_d = nc.dram_tensor("b_qk", [128, 4], F32, kind="ExternalInput")
    id_d = nc.dram_tensor("ident", [128, 128], BF16, kind="ExternalInput")
    bv_d = nc.dram_tensor("b_v128", [128, DQK], F32, kind="ExternalInput")
    bo_d = nc.dram_tensor("b_o128", [128, DIN], F32, kind="ExternalInput")
    out_d = nc.dram_tensor("out_partial", [S, DIN], BF16,
                           kind="ExternalOutput")

    with tile.TileContext(nc) as tc:
        with (
            tc.tile_pool(name="persist", bufs=1) as pp,
            tc.tile_pool(name="expp", bufs=49) as ep,         # exp outputs
            tc.tile_pool(name="recp", bufs=4) as rp,
            tc.tile_pool(name="outp", bufs=4) as op_,
            tc.tile_pool(name="ps_s", bufs=2, space="PSUM") as pss,
            tc.tile_pool(name="ps_q", bufs=1, space="PSUM") as pq,
            tc.tile_pool(name="ps_v", bufs=2, space="PSUM") as ppv,
            tc.tile_pool(name="ps_o", bufs=1, space="PSUM") as pso,
        ):
            # ---- persistent SBUF tensors (all-bf16 data path) ----
            xt = pp.tile([128, KC * S], BF16, tag="xt")          # x^T  [c][s]
            wqkT = pp.tile([128, 4 * DIN], BF16, tag="wqkT")     # [blk][c][j]
            wvT = pp.tile([128, KC * DQK], BF16, tag="wvT")      # [c][dout]
            woT = pp.tile([128, 2 * DIN], BF16, tag="woT")       # [ch][dm]
            qk = pp.tile([128, 4 * S], BF16, tag="qk")           # q0,q1,k0,k1
            vv = pp.tile([128, SB * HPC * V65], BF16, tag="vv")  # V' blocks
            at = pp.tile([128, SB * DQK], BF16, tag="at")        # attn out
            atT = pp.tile([128, 2 * S], BF16, tag="atT")         # at^T
            bqk = pp.tile([128, 4], F32, tag="bqk")
            ident = pp.tile([128, 128], BF16, tag="ident")
            bv128 = pp.tile([128, DQK], F32, tag="bv128")
            bo128 = pp.tile([128, DIN], F32, tag="bo128")
            ones1 = pp.tile([1, 128], BF16, tag="ones1")
            bo_bf = pp.tile([1, DIN], BF16, tag="bo_bf")

            xt3 = xt[:].rearrange("p (c s) -> p c s", c=KC)
            wqkT3 = wqkT[:].rearrange("p (b c j) -> p b c j", b=4, c=KC)
            wvT3 = wvT[:].rearrange("p (c d) -> p c d", c=KC)
            woT3 = woT[:].rearrange("p (h d) -> p h d", h=2)
            qk3 = qk[:].rearrange("p (b s) -> p b s", b=4)
            vv4 = vv[:].rearrange("p (i h d) -> p i h d", i=SB, h=HPC)
            at3 = at[:].rearrange("p (i d) -> p i d", i=SB)
            atT3 = atT[:].rearrange("p (h s) -> p h s", h=2)

            # ---- PE warm-up: skinny [128,128] matmul train ----
            # The cost model's p-state clock reaches full speed only after
            # ~3us of near-continuous PE activity; a train of narrow matmuls
            # (107ns each at mid clock) spanning t~0.2-5.5us ramps the clock
            # on ~2.5us of fake work so the first real projection (~5.6us,
            # gated on the x/w DMAs) runs at full speed immediately.
            warm = pp.tile([128, 128], BF16, tag="warm")
            nc.vector.memset(warm[:], 0.0)
            for _ in range(40):
                psw = pss.tile([128, QC], F32, tag="pss")
                nc.tensor.matmul(psw[:, 0:128], warm[:], warm[:],
                                 start=True, stop=True)

            # ---- small loads ----
            for i in range(SB):
                nc.vector.memset(vv4[:, i, :, DK:V65], 1.0)
            # (bv128/bo128 loaded after the bulk weights/x below)

            # ---- bulk loads, ordered for earliest first score ----
            def dma_xt(k2):
                # two s-blocks per DMA (keeps >=512B contiguity)
                nc.sync.dma_start(
                    xt3[:, :, k2 * 256:(k2 + 1) * 256],
                    xt_d.ap()[k2 * 128:(k2 + 1) * 128, :])

            def dma_wqkT(b):
                nc.sync.dma_start(wqkT[:, b * DIN:(b + 1) * DIN],
                                  wqkT_d.ap()[:, b * DIN:(b + 1) * DIN])

            # head DMAs split across the SP and ACT HWDGE queues: one
            # queue issues a descriptor only every ~650ns, which otherwise
            # serializes the transfers the first projections wait on
            def dma_xt_on(eng, k2):
                eng.dma_start(xt3[:, :, k2 * 256:(k2 + 1) * 256],
                              xt_d.ap()[k2 * 128:(k2 + 1) * 128, :])

            dma_wqkT(0)
            dma_xt_on(nc.sync, 0)
            nc.sync.dma_start(bqk[:], bqk_d.ap())
            dma_xt_on(nc.scalar, 1)
            nc.sync.dma_start(wqkT[:, 2 * DIN:3 * DIN],
                              wqkT_d.ap()[:, 2 * DIN:3 * DIN])
            dma_xt_on(nc.scalar, 3)
            dma_xt_on(nc.sync, 2)
            for k2 in range(4, 8):
                dma_xt_on(nc.sync, k2)
            nc.sync.dma_start(wvT[:], wvT_d.ap())
            dma_wqkT(1)
            dma_wqkT(3)
            nc.sync.dma_start(bv128[:], bv_d.ap())
            nc.sync.dma_start(woT[:], woT_d.ap())
            nc.sync.dma_start(bo128[:], bo_d.ap())
            nc.sync.dma_start(ident[:], id_d.ap())
            nc.vector.memset(ones1[:], 1.0)
            nc.vector.tensor_copy(bo_bf[:], bo128[0:1, :])

            # ---- projections (bf16 matmuls, bf16 outputs) ----
            def proj_qkT(dblk, sc):
                # Q^T/K^T block dblk over s columns [sc*512, (sc+1)*512)
                ps = pss.tile([128, QC], F32, tag="pss")
                for c in range(KC):
                    nc.tensor.matmul(
                        ps[:, 0:512],
                        wqkT3[:, dblk, c, :],
                        xt3[:, c, sc * 512:(sc + 1) * 512],
                        start=(c == 0), stop=(c == KC - 1))
                nc.vector.tensor_scalar_add(
                    qk3[:, dblk, sc * 512:(sc + 1) * 512],
                    ps[:, 0:512], bqk[:, dblk:dblk + 1])

            proj_qkT(0, 0)
            proj_qkT(2, 0)
            proj_qkT(0, 1)

            # ---- attention machinery ----
            def scores_exp(qc, h, drain, half_n2=None):
                """scores+exp for head h over q cols [qc*QC,(qc+1)*QC)
                (or the 512-col half half_n2 of that range); after each
                slab's activation, drain() weaves fill work from the global
                queue up to the slab's PE budget."""
                pr, hl = divmod(h, 2)
                qblk, kblk = pr, 2 + pr
                exs = []
                for i in range(SB):
                    ps = pss.tile([128, QC], F32, tag="pss")
                    if half_n2 is None:
                        for n2 in range(QC // 512):
                            nc.tensor.matmul(
                                ps[:, n2 * 512:(n2 + 1) * 512],
                                qk3[64 * hl:64 * hl + 64, kblk,
                                    i * 128:(i + 1) * 128],
                                qk3[64 * hl:64 * hl + 64, qblk,
                                    qc * QC + n2 * 512:
                                    qc * QC + (n2 + 1) * 512],
                                start=True, stop=True)
                        ex = ep.tile([128, QC], BF16, tag="ex")
                        nc.scalar.activation(ex[:], ps[:], AF.Exp,
                                             scale=SCALE_INV)
                        drain(612)
                    else:
                        nc.tensor.matmul(
                            ps[:, 0:512],
                            qk3[64 * hl:64 * hl + 64, kblk,
                                i * 128:(i + 1) * 128],
                            qk3[64 * hl:64 * hl + 64, qblk,
                                qc * QC + half_n2 * 512:
                                qc * QC + (half_n2 + 1) * 512],
                            start=True, stop=True)
                        ex = ep.tile([128, QC], BF16, tag="ex")
                        nc.scalar.activation(ex[:, 0:512], ps[:, 0:512],
                                             AF.Exp, scale=SCALE_INV)
                        drain(440)
                    exs.append(ex)
                return exs

            def pv_one(qc, h, exs, qb, with_atT=False, half=False,
                       pool=None):
                # PV (transposed) + normalize for one q-block of head h.
                # half: exs hold 512 q-cols; qb local 0-3 within the half.
                sblk = qc * SBH + qb
                qoff = (qb % 4) * 128 if half else qb * 128
                pl, tg = pool if pool else (ppv, "ppv")
                po = pl.tile([128, 512], F32, tag=tg, name="po")
                for i in range(SB):
                    nc.tensor.matmul(
                        po[:, 0:V65],
                        exs[i][:, qoff:qoff + 128],
                        vv4[:, i, h, :],
                        start=(i == 0), stop=(i == SB - 1))
                rec = rp.tile([128, 1], F32, tag="rec")
                nc.vector.reciprocal(rec[:], po[:, DK:V65])
                nc.vector.tensor_scalar_mul(
                    at3[:, sblk, h * DK:(h + 1) * DK],
                    po[:, 0:DK], rec[:])
                if with_atT:
                    nc.sync.dma_start_transpose(
                        atT3[:, :, sblk * 128:(sblk + 1) * 128],
                        at3[:, sblk, :])

            # ---- fill-work thunk factories (costs ~= full-speed PE ns) ----
            proj_ctr = [0]

            def qkp4(dblk, sc):
                """proj_qkT as 4 thunks of 2 contraction chunks (~426ns).
                Accumulators alternate the pq and pso banks (pso is idle
                until the first out-projections in P7, and projections end
                by P5), so consecutive sets never serialize through one
                bank's write-after-read on the DVE bias-add."""
                box = []
                npj = proj_ctr[0]
                proj_ctr[0] += 1
                ppool, ptag = (pq, "pq") if npj % 2 == 0 else (pso, "pso")

                def mk(j):
                    def t():
                        if j == 0:
                            box.append(ppool.tile([128, 512], F32, tag=ptag,
                                                  name="pqt"))
                        ps = box[0]
                        for c in (2 * j, 2 * j + 1):
                            nc.tensor.matmul(
                                ps[:],
                                wqkT3[:, dblk, c, :],
                                xt3[:, c, sc * 512:(sc + 1) * 512],
                                start=(c == 0), stop=(c == KC - 1))
                        if j == 3:
                            nc.vector.tensor_scalar_add(
                                qk3[:, dblk, sc * 512:(sc + 1) * 512],
                                ps[:], bqk[:, dblk:dblk + 1])
                    return (426, t)
                return [mk(j) for j in range(4)]

            def vph(i):
                """proj_v for s-block i as 2 thunks of 4 chunks (~427ns)."""
                box = []

                def mk(j):
                    def t():
                        if j == 0:
                            box.append(ppv.tile([128, 512], F32, tag="ppv", name="vpt"))
                        ps = box[0]
                        for c in range(4 * j, 4 * j + 4):
                            nc.tensor.matmul(
                                ps[:, 0:DQK],
                                xt3[:, c, i * 128:(i + 1) * 128],
                                wvT3[:, c, :],
                                start=(c == 0), stop=(c == KC - 1))
                        if j == 1:
                            nc.vector.tensor_add(
                                vv4[:, i, :, 0:DK],
                                ps[:, 0:DQK].rearrange(
                                    "p (h d) -> p h d", h=HPC),
                                bv128[:].rearrange("p (h d) -> p h d", h=HPC))
                    return (427, t)
                return [mk(j) for j in range(2)]

            def pvset(qc, h, exs_ref, with_atT=False, rotate=False):
                """8 PV thunks (one per q-block, ~433ns each). rotate:
                spread accumulators over ppv+pq+pso (only valid where the
                projection/out-proj banks are idle, i.e. pass P6) so
                back-to-back PV units don't serialize on ppv's two bufs."""
                cyc = [None, None, (pq, "pq"), (pso, "pso")]

                def mk(qb):
                    pool = cyc[qb % 4] if rotate else None
                    return (433, lambda: pv_one(qc, h, exs_ref(), qb,
                                                with_atT=with_atT,
                                                pool=pool))
                return [mk(qb) for qb in range(SBH)]

            op_ctr = [0]

            def oph(sblk):
                """out-projection for s-block as 2 thunks (~426ns each).
                PSUM alternates between the pso bank and the pq bank (idle
                after the projections finish) so consecutive halves don't
                serialize on one bank; the bias-add alternates DVE/Pool so
                neither engine paces the op stream."""
                box = []

                def mk(dmh):
                    def t():
                        if dmh == 0:
                            box.append(op_.tile([128, DIN], BF16, tag="ot",
                                                name="ott"))
                        ot = box[0]
                        n = op_ctr[0]
                        op_ctr[0] += 1
                        pool = pso if n % 2 == 0 else pq
                        ps = pool.tile([128, 512], F32,
                                       tag="pso" if n % 2 == 0 else "pq",
                                       name="opps")
                        for ch in range(2):
                            nc.tensor.matmul(
                                ps[:],
                                atT3[:, ch, sblk * 128:(sblk + 1) * 128],
                                woT3[:, ch, dmh * 512:(dmh + 1) * 512],
                                start=(ch == 0), stop=(ch == 1))
                        nc.vector.tensor_add(
                            ot[:, dmh * 512:(dmh + 1) * 512], ps[:],
                            bo128[:, dmh * 512:(dmh + 1) * 512])
                        if dmh == 1:
                            nc.sync.dma_start(
                                out_d.ap()[sblk * 128:(sblk + 1) * 128, :],
                                ot[:])
                    return (426, t)
                return [mk(0), mk(1)]

            # ---- global fill queue + drain ----
            from collections import deque
            work_q = deque()
            credit = [0.0]

            def drain(budget):
                avail = budget + credit[0]
                while work_q and avail >= 0.5 * work_q[0][0]:
                    c, fn = work_q.popleft()
                    fn()
                    avail -= c
                credit[0] = min(max(avail, 0.0), 100.0)

            def push(*thunk_lists):
                for tl in thunk_lists:
                    work_q.extend(tl)

            # ---- pass schedule ----
            # P1=(h0,qc0) P2=(h0,qc1) P3=(h1,qc0) P4=(h1,qc1)
            # P5=(h2,qc0) P6=(h3,qc0) P7=(h2,qc1) P8=(h3,qc1) in halves.
            # h0/h1 share q0/k0 projections; h2/h3 share q1/k1 — the hybrid
            # order spreads projection deadlines and keeps qc0 out-proj off
            # the tail.
            push(qkp4(2, 1), qkp4(2, 2), qkp4(2, 3), qkp4(0, 2), qkp4(0, 3))
            ex_p1 = scores_exp(0, 0, drain)
            for i in range(SB):
                push(vph(i))
            ex_p2 = scores_exp(1, 0, drain)
            push(pvset(0, 0, lambda: ex_p1))
            push(qkp4(1, 0), qkp4(1, 1))
            ex_p3 = scores_exp(0, 1, drain)
            push(qkp4(3, 0), qkp4(3, 1), qkp4(3, 2), qkp4(3, 3))
            push(pvset(1, 0, lambda: ex_p2))
            ex_p4 = scores_exp(1, 1, drain)
            push(qkp4(1, 2), qkp4(1, 3))
            push(pvset(0, 1, lambda: ex_p3))
            ex_p5 = scores_exp(0, 2, drain)
            push(pvset(1, 1, lambda: ex_p4, rotate=True))
            push(pvset(0, 2, lambda: ex_p5, rotate=True))
            ex_p6 = scores_exp(0, 3, drain)
            push(pvset(0, 3, lambda: ex_p6, with_atT=True))
            for k in range(SBH):
                push(oph(k))
            ex_p7 = scores_exp(1, 2, drain)
            push(pvset(1, 2, lambda: ex_p7))
            ex_p8a = scores_exp(1, 3, drain, half_n2=0)
            pvh_a = [t for _, t in pvset(1, 3, lambda: ex_p8a,
                                         with_atT=True)][:4]
            push([(433, pvh_a[0]), (433, pvh_a[1])],
                 oph(SBH), [(433, pvh_a[2])],
                 oph(SBH + 1), [(433, pvh_a[3])],
                 oph(SBH + 2), oph(SBH + 3))
            ex_p8b = scores_exp(1, 3, drain, half_n2=1)

            def warm1():
                # p-state keep-alive during the dependency-latency-bound
                # tail (pss is free once the last score slab is done)
                psw = pss.tile([128, QC], F32, tag="pss", name="pswt")
                nc.tensor.matmul(psw[:, 0:128], warm[:], warm[:],
                                 start=True, stop=True)

            def pv_tail(qb):
                # final-head PV + PE-transpose of the at row (the serial
                # DMA-transpose path costs ~1.8us per block in the tail)
                pv_one(1, 3, ex_p8b, qb, half=True)
                sblk = SBH + qb
                pst = pss.tile([128, QC], F32, tag="pss", name="pstt")
                pstb = pst.bitcast(BF16)
                for ch in range(2):
                    nc.tensor.matmul(
                        pstb[:, ch * 128:(ch + 1) * 128],
                        at3[:, sblk, ch * 128:(ch + 1) * 128],
                        ident[:], is_transpose=True,
                        start=(ch == 0), stop=(ch == 1),
                        skip_group_check=True)
                nc.vector.tensor_copy(
                    atT3[:, :, sblk * 128:(sblk + 1) * 128],
                    pstb[:, 0:256].rearrange("p (h j) -> p h j", h=2))


            def oph_tail(sblk, dmh, on_act, use_ppv=False):
                # tail out-proj half; on_act folds the bias in via a 1-deep
                # ones x b_out matmul and copies PSUM->SBUF on the (idle)
                # ACT engine so DVE and ACT each carry half the tail stream
                ot = ot_tail[sblk - SBH - 4]
                if use_ppv:
                    # ppv is idle once the last PV unit is done; a third
                    # bank removes the pso/pq write-after-read waits from
                    # the final op halves
                    ps = ppv.tile([128, 512], F32, tag="ppv", name="opts")
                else:
                    n = op_ctr[0]
                    op_ctr[0] += 1
                    pool = pso if n % 2 == 0 else pq
                    ps = pool.tile([128, 512], F32,
                                   tag="pso" if n % 2 == 0 else "pq",
                                   name="opts")
                if on_act:
                    nc.tensor.matmul(
                        ps[:], ones1[:], bo_bf[0:1,
                                               dmh * 512:
                                               (dmh + 1) * 512],
                        start=True, stop=False, skip_group_check=True)
                for ch in range(2):
                    nc.tensor.matmul(
                        ps[:],
                        atT3[:, ch, sblk * 128:(sblk + 1) * 128],
                        woT3[:, ch, dmh * 512:(dmh + 1) * 512],
                        start=(not on_act and ch == 0), stop=(ch == 1),
                        skip_group_check=True)
                if on_act:
                    nc.scalar.activation(
                        ot[:, dmh * 512:(dmh + 1) * 512], ps[:],
                        AF.Copy)
                else:
                    nc.vector.tensor_add(
                        ot[:, dmh * 512:(dmh + 1) * 512], ps[:],
                        bo128[:, dmh * 512:(dmh + 1) * 512])
                # stream each half out as soon as its bias is applied:
                # the final three full-row DMAs otherwise serialize 2.2us
                # of transfer after the last compute
                nc.sync.dma_start(
                    out_d.ap()[sblk * 128:(sblk + 1) * 128,
                               dmh * 512:(dmh + 1) * 512],
                    ot[:, dmh * 512:(dmh + 1) * 512])

            # tail: drain leftovers, then final PV half (PE transpose) +
            # out-proj with ACT/DVE alternation; warm matmuls keep the PE
            # clock at full speed across dependency stalls
            ot_tail = [op_.tile([128, DIN], BF16, tag="ot",
                                name=f"ott{k}") for k in range(4)]
            pv_tail(4)
            pv_tail(5)
            drain(10**9)
            oph_tail(SBH + 4, 0, True)
            pv_tail(6)
            oph_tail(SBH + 4, 1, False)
            pv_tail(7)
            oph_tail(SBH + 5, 0, True)
            oph_tail(SBH + 5, 1, False)
            oph_tail(SBH + 6, 0, True, use_ppv=True)
            oph_tail(SBH + 6, 1, False)
            oph_tail(SBH + 7, 0, True, use_ppv=True)
            oph_tail(SBH + 7, 1, False)

    nc.compile()
    return nc


def shard_inputs(x, w_qkv, b_qkv, w_out, b_out, cfg: Cfg = FULL):
    """Build the 8 per-core input maps from full inputs (host-side layout
    marshaling: transpose/reshape/stack/dtype-cast, no arithmetic)."""
    DIN, DQK, KC, S = cfg.DIN, cfg.DQK, cfg.KC, cfg.S
    D = DIN
    bf16 = mybir.dt.np(mybir.dt.bfloat16)
    x = np.asarray(x, dtype=np.float32)
    w_qkv = np.asarray(w_qkv, dtype=np.float32)
    b_qkv = np.asarray(b_qkv, dtype=np.float32)
    w_out = np.asarray(w_out, dtype=np.float32)
    b_out = np.asarray(b_out, dtype=np.float32)
    zeros_bo = np.zeros((128, DIN), dtype=np.float32)
    bo128 = np.ascontiguousarray(
        np.broadcast_to(b_out.reshape(1, DIN), (128, DIN)))

    # x^T images per batch, as 2-s-block slabs:
    # row k2*128+p, col c*256+j = x[k2*256+j, c*128+p]
    xt_imgs = []
    for b in range(2):
        arr = x[b].astype(bf16).reshape(8, 256, KC, 128)  # (k2, j, c, p)
        xt_imgs.append(np.ascontiguousarray(
            arr.transpose(0, 3, 2, 1).reshape(8 * 128, S)))

    in_maps = []
    for c in range(cfg.N_CORES):
        b, hg = divmod(c, 4)
        sl = slice(hg * DQK, (hg + 1) * DQK)
        wq = w_qkv[0 * D:1 * D][sl]
        wk = w_qkv[1 * D:2 * D][sl]
        wv = w_qkv[2 * D:3 * D][sl]
        wo = w_out[:, sl]
        bq = b_qkv[0 * D:1 * D][sl]
        bk = b_qkv[1 * D:2 * D][sl]
        bqk_np = np.stack([bq[0:128], bq[128:256],
                           bk[0:128], bk[128:256]], axis=1)
        # w_qkT image [128, 4*1024]: col b*1024+c*128+j = W[b*128+j, c*128+p]
        wqk = np.concatenate([wq, wk], axis=0).astype(bf16)  # [512, 1024]
        wqkT = (wqk.reshape(4, 128, KC, 128)            # (blk, j, c, p)
                .transpose(3, 0, 2, 1).reshape(128, 4 * DIN))
        # w_vT image [128, 8*256]: col c*256+d = Wv[d, c*128+p]
        wvT = (wv.astype(bf16).reshape(DQK, KC, 128)    # (d, c, p)
               .transpose(2, 1, 0).reshape(128, KC * DQK))
        # w_oT image [128, 2*1024]: col ch*1024+dm = Wo[dm, ch*128+p]
        woT = (wo.astype(bf16).reshape(DIN, 2, 128)     # (dm, ch, p)
               .transpose(2, 1, 0).reshape(128, 2 * DIN))
        bv128 = np.broadcast_to(
            b_qkv[2 * D:3 * D][sl].reshape(1, DQK), (128, DQK))
        in_maps.append({
            "ident": np.eye(128, dtype=bf16),
            "xt": xt_imgs[b],
            "w_qkT": np.ascontiguousarray(wqkT),
            "w_vT": np.ascontiguousarray(wvT),
            "w_oT": np.ascontiguousarray(woT),
            "b_qk": np.ascontiguousarray(bqk_np),
            "b_v128": np.ascontiguousarray(bv128),
            "b_o128": bo128 if hg == 0 else zeros_bo,
        })
    return in_maps


def gather_output(results, cfg: Cfg = FULL):
    outs = []
    for b in range(2):
        acc = results[4 * b]["out_partial"].astype(np.float32)
        for c in range(4 * b + 1, 4 * b + 4):
            acc = acc + results[c]["out_partial"].astype(np.float32)
        outs.append(acc)
    return np.stack(outs, axis=0)


_NC_CACHE = {}


def _get_nc(cfg: Cfg = FULL):
    if cfg not in _NC_CACHE:
        _NC_CACHE[cfg] = build_nc(cfg)
    return _NC_CACHE[cfg]


def kernel(x, w_qkv, b_qkv, w_out, b_out):
    cfg = FULL
    nc = _get_nc(cfg)
    in_maps = shard_inputs(x, w_qkv, b_qkv, w_out, b_out, cfg)
    res = run_bass_kernel_spmd(nc, in_maps, core_ids=list(range(cfg.N_CORES)))
    return gather_output(res.results, cfg)


if __name__ == "__main__":
    rng = np.random.default_rng(0)
    D = FULL.DIN
    x = rng.standard_normal((2, FULL.S, D), dtype=np.float32)
    w_qkv = (rng.standard_normal((3 * D, D), dtype=np.float32) / np.sqrt(D))
    b_qkv = rng.standard_normal(3 * D, dtype=np.float32) * 0.02
    w_out = rng.standard_normal((D, D), dtype=np.float32) / np.sqrt(D)
    b_out = rng.standard_normal(D, dtype=np.float32) * 0.02
    out = kernel(x=x, w_qkv=w_qkv, b_qkv=b_qkv, w_out=w_out, b_out=b_out)
    print("out", out.shape, out.dtype, float(np.abs(out).mean()))



# revision 59
# speedup vs baseline: 1.0029x; 1.0029x over previous
"""Trainium2 Bass kernel for 16-head MultiHeadAttention (B=2, S=2048, D=1024).

Sharding: 8 cores = 2 (batch) x 4 (head groups of 4 heads).
Each core computes the qkv projection for its 4 heads, attention, and a
partial out-projection (TP over heads); the host sums the 4 partials per
batch element.

On-device schedule (~163us/core; PE floor ~137us, ACT exp floor ~128us):
  - Softmax-exp streams on the ACT engine as 8 passes of 16 PSUM score
    slabs [128keys x 1024q] (last pass split into 512-col halves so its
    PV/out-proj can start early). Scores for slab i+2 overlap exp of i.
  - All other PE work (q/k projections split into 2-chunk quarters,
    v-projection halves, PV units, out-proj halves) lives in a global
    FIFO drained after each slab against the ACT pace (612ns/slab fill
    budget, 100ns carry cap), so the PE never outruns or starves ACT.
  - Pass order (h0,qc0)(h0,qc1)(h1,qc0)(h1,qc1)(h2,qc0)(h3,qc0)(h2,qc1)
    (h3,qc1): h0/h1 share q0/k0 and h2/h3 share q1/k1 projections, which
    spreads projection deadlines; qc0 out-projections run mid-stream.
  - PV in transposed orientation (out [q=128, dk+1]) with the softmax
    denominator via a ones-column in V'; normalize = DVE reciprocal +
    tensor_scalar_mul; at rows -> at^T by DMA transpose mid-stream and
    by PE transpose in the tail (serial-DMA latency would gate the end).
  - Tail out-proj halves alternate DVE bias-add and [1-deep ones x b_out
    bias matmul + ACT copy] so DVE and the (by then idle) ACT engine
    each carry half; their PSUM alternates the pso/pq banks (plus the
    ppv bank, idle after the final PV unit, for the last blocks). Projection
    accumulators alternate those same two banks mid-stream (disjoint
    lifetimes) so consecutive projections don't serialize through one
    bank's write-after-read on the bias-add.
  - The cost model's PE clock ramps over ~3us of activity and drops
    after long idles: a skinny [128,128] warm-up matmul train covers the
    initial DMA window so the first projections run at full speed.
  - Head DMAs are split across the SP and ACT HWDGE queues and ordered
    so the first projections' inputs land first.
"""

import sys
from dataclasses import dataclass

for _p in ("/opt/trn_rl_repo",):
    if _p not in sys.path:
        sys.path.insert(0, _p)

import numpy as np

import concourse.bass as bass  # noqa: E402,F401
import concourse.bacc as bacc  # noqa: E402
import concourse.tile as tile  # noqa: E402
from concourse import mybir  # noqa: E402
from concourse.bass_utils import run_bass_kernel_spmd  # noqa: E402

F32 = mybir.dt.float32
BF16 = mybir.dt.bfloat16
AF = mybir.ActivationFunctionType


@dataclass(frozen=True)
class Cfg:
    S: int = 2048      # sequence length
    DIN: int = 1024    # model dim
    HPC: int = 4       # heads per core
    DK: int = 64       # head dim
    N_CORES: int = 8

    @property
    def DQK(self):
        return self.HPC * self.DK  # 256 per-core q/k/v width

    @property
    def KC(self):
        return self.DIN // 128     # 8 contraction chunks

    @property
    def SB(self):
        return self.S // 128       # 16 sequence blocks


FULL = Cfg()


def build_nc(cfg: Cfg = FULL):
    S, DIN, HPC, DK = cfg.S, cfg.DIN, cfg.HPC, cfg.DK
    DQK, KC, SB = cfg.DQK, cfg.KC, cfg.SB
    QC = 1024                 # q-chunk width for attention passes
    SBH = QC // 128           # 8 s-blocks per pass
    SCALE_INV = 1.0 / float(np.sqrt(DK))
    V65 = DK + 1              # V' width per head (denominator ones col)

    nc = bacc.Bacc("TRN2", target_bir_lowering=False, debug=False,
                   num_devices=cfg.N_CORES)

    # x^T in bf16, delivered as 2-s-block slabs: row k2*128+p holds
    # x[k2*256+j, c*128+p] at col c*256+j (see shard_inputs) so each
    # [128,2048] DMA delivers all 8 c-chunks of two s-blocks contiguously.
    xt_d = nc.dram_tensor("xt", [8 * 128, S], BF16, kind="ExternalInput")
    wqkT_d = nc.dram_tensor("w_qkT", [128, 4 * DIN], BF16,
                            kind="ExternalInput")
    wvT_d = nc.dram_tensor("w_vT", [128, KC * DQK], BF16,
                           kind="ExternalInput")
    woT_d = nc.dram_tensor("w_oT", [128, 2 * DIN], BF16,
                           kind="ExternalInput")
    bqk_d = nc.dram_tensor("b_qk", [128, 4], F32, kind="ExternalInput")
    id_d = nc.dram_tensor("ident", [128, 128], BF16, kind="ExternalInput")
    bv_d = nc.dram_tensor("b_v128", [128, DQK], F32, kind="ExternalInput")
    bo_d = nc.dram_tensor("b_o128", [128, DIN], F32, kind="ExternalInput")
    out_d = nc.dram_tensor("out_partial", [S, DIN], BF16,
                           kind="ExternalOutput")

    with tile.TileContext(nc) as tc:
        with (
            tc.tile_pool(name="persist", bufs=1) as pp,
            tc.tile_pool(name="expp", bufs=49) as ep,         # exp outputs
            tc.tile_pool(name="recp", bufs=4) as rp,
            tc.tile_pool(name="outp", bufs=4) as op_,
            tc.tile_pool(name="ps_s", bufs=2, space="PSUM") as pss,
            tc.tile_pool(name="ps_q", bufs=1, space="PSUM") as pq,
            tc.tile_pool(name="ps_v", bufs=2, space="PSUM") as ppv,
            tc.tile_pool(name="ps_o", bufs=1, space="PSUM") as pso,
        ):
            # ---- persistent SBUF tensors (all-bf16 data path) ----
            xt = pp.tile([128, KC * S], BF16, tag="xt")          # x^T  [c][s]
            wqkT = pp.tile([128, 4 * DIN], BF16, tag="wqkT")     # [blk][c][j]
            wvT = pp.tile([128, KC * DQK], BF16, tag="wvT")      # [c][dout]
            woT = pp.tile([128, 2 * DIN], BF16, tag="woT")       # [ch][dm]
            qk = pp.tile([128, 4 * S], BF16, tag="qk")           # q0,q1,k0,k1
            vv = pp.tile([128, SB * HPC * V65], BF16, tag="vv")  # V' blocks
            at = pp.tile([128, SB * DQK], BF16, tag="at")        # attn out
            atT = pp.tile([128, 2 * S], BF16, tag="atT")         # at^T
            bqk = pp.tile([128, 4], F32, tag="bqk")
            ident = pp.tile([128, 128], BF16, tag="ident")
            bv128 = pp.tile([128, DQK], F32, tag="bv128")
            bo128 = pp.tile([128, DIN], F32, tag="bo128")
            ones1 = pp.tile([1, 128], BF16, tag="ones1")
            bo_bf = pp.tile([1, DIN], BF16, tag="bo_bf")

            xt3 = xt[:].rearrange("p (c s) -> p c s", c=KC)
            wqkT3 = wqkT[:].rearrange("p (b c j) -> p b c j", b=4, c=KC)
            wvT3 = wvT[:].rearrange("p (c d) -> p c d", c=KC)
            woT3 = woT[:].rearrange("p (h d) -> p h d", h=2)
            qk3 = qk[:].rearrange("p (b s) -> p b s", b=4)
            vv4 = vv[:].rearrange("p (i h d) -> p i h d", i=SB, h=HPC)
            at3 = at[:].rearrange("p (i d) -> p i d", i=SB)
            atT3 = atT[:].rearrange("p (h s) -> p h s", h=2)

            # ---- PE warm-up: skinny [128,128] matmul train ----
            # The cost model's p-state clock reaches full speed only after
            # ~3us of near-continuous PE activity; a train of narrow matmuls
            # (107ns each at mid clock) spanning t~0.2-5.5us ramps the clock
            # on ~2.5us of fake work so the first real projection (~5.6us,
            # gated on the x/w DMAs) runs at full speed immediately.
            warm = pp.tile([128, 128], BF16, tag="warm")
            nc.vector.memset(warm[:], 0.0)
            for _ in range(40):
                psw = pss.tile([128, QC], F32, tag="pss")
                nc.tensor.matmul(psw[:, 0:128], warm[:], warm[:],
                                 start=True, stop=True)

            # ---- small loads ----
            for i in range(SB):
                nc.vector.memset(vv4[:, i, :, DK:V65], 1.0)
            # (bv128/bo128 loaded after the bulk weights/x below)

            # ---- bulk loads, ordered for earliest first score ----
            def dma_xt(k2):
                # two s-blocks per DMA (keeps >=512B contiguity)
                nc.sync.dma_start(
                    xt3[:, :, k2 * 256:(k2 + 1) * 256],
                    xt_d.ap()[k2 * 128:(k2 + 1) * 128, :])

            def dma_wqkT(b):
                nc.sync.dma_start(wqkT[:, b * DIN:(b + 1) * DIN],
                                  wqkT_d.ap()[:, b * DIN:(b + 1) * DIN])

            # head DMAs split across the SP and ACT HWDGE queues: one
            # queue issues a descriptor only every ~650ns, which otherwise
            # serializes the transfers the first projections wait on
            def dma_xt_on(eng, k2):
                eng.dma_start(xt3[:, :, k2 * 256:(k2 + 1) * 256],
                              xt_d.ap()[k2 * 128:(k2 + 1) * 128, :])

            dma_wqkT(0)
            dma_xt_on(nc.sync, 0)
            nc.sync.dma_start(bqk[:], bqk_d.ap())
            dma_xt_on(nc.scalar, 1)
            nc.sync.dma_start(wqkT[:, 2 * DIN:3 * DIN],
                              wqkT_d.ap()[:, 2 * DIN:3 * DIN])
            dma_xt_on(nc.scalar, 3)
            dma_xt_on(nc.sync, 2)
            for k2 in range(4, 8):
                dma_xt_on(nc.sync, k2)
            nc.sync.dma_start(wvT[:], wvT_d.ap())
            dma_wqkT(1)
            dma_wqkT(3)
            nc.sync.dma_start(bv128[:], bv_d.ap())
            nc.sync.dma_start(woT[:], woT_d.ap())
            nc.sync.dma_start(bo128[:], bo_d.ap())
            nc.sync.dma_start(ident[:], id_d.ap())
            nc.vector.memset(ones1[:], 1.0)
            nc.vector.tensor_copy(bo_bf[:], bo128[0:1, :])

            # ---- projections (bf16 matmuls, bf16 outputs) ----
            def proj_qkT(dblk, sc):
                # Q^T/K^T block dblk over s columns [sc*512, (sc+1)*512)
                ps = pss.tile([128, QC], F32, tag="pss")
                for c in range(KC):
                    nc.tensor.matmul(
                        ps[:, 0:512],
                        wqkT3[:, dblk, c, :],
                        xt3[:, c, sc * 512:(sc + 1) * 512],
                        start=(c == 0), stop=(c == KC - 1))
                nc.vector.tensor_scalar_add(
                    qk3[:, dblk, sc * 512:(sc + 1) * 512],
                    ps[:, 0:512], bqk[:, dblk:dblk + 1])

            proj_qkT(0, 0)
            proj_qkT(2, 0)
            proj_qkT(0, 1)

            # ---- attention machinery ----
            def scores_exp(qc, h, drain, half_n2=None):
                """scores+exp for head h over q cols [qc*QC,(qc+1)*QC)
                (or the 512-col half half_n2 of that range); after each
                slab's activation, drain() weaves fill work from the global
                queue up to the slab's PE budget."""
                pr, hl = divmod(h, 2)
                qblk, kblk = pr, 2 + pr
                exs = []
                for i in range(SB):
                    ps = pss.tile([128, QC], F32, tag="pss")
                    if half_n2 is None:
                        for n2 in range(QC // 512):
                            nc.tensor.matmul(
                                ps[:, n2 * 512:(n2 + 1) * 512],
                                qk3[64 * hl:64 * hl + 64, kblk,
                                    i * 128:(i + 1) * 128],
                                qk3[64 * hl:64 * hl + 64, qblk,
                                    qc * QC + n2 * 512:
                                    qc * QC + (n2 + 1) * 512],
                                start=True, stop=True)
                        ex = ep.tile([128, QC], BF16, tag="ex")
                        nc.scalar.activation(ex[:], ps[:], AF.Exp,
                                             scale=SCALE_INV)
                        drain(612)
                    else:
                        nc.tensor.matmul(
                            ps[:, 0:512],
                            qk3[64 * hl:64 * hl + 64, kblk,
                                i * 128:(i + 1) * 128],
                            qk3[64 * hl:64 * hl + 64, qblk,
                                qc * QC + half_n2 * 512:
                                qc * QC + (half_n2 + 1) * 512],
                            start=True, stop=True)
                        ex = ep.tile([128, QC], BF16, tag="ex")
                        nc.scalar.activation(ex[:, 0:512], ps[:, 0:512],
                                             AF.Exp, scale=SCALE_INV)
                        drain(440)
                    exs.append(ex)
                return exs

            def pv_one(qc, h, exs, qb, with_atT=False, half=False,
                       pool=None):
                # PV (transposed) + normalize for one q-block of head h.
                # half: exs hold 512 q-cols; qb local 0-3 within the half.
                sblk = qc * SBH + qb
                qoff = (qb % 4) * 128 if half else qb * 128
                pl, tg = pool if pool else (ppv, "ppv")
                po = pl.tile([128, 512], F32, tag=tg, name="po")
                for i in range(SB):
                    nc.tensor.matmul(
                        po[:, 0:V65],
                        exs[i][:, qoff:qoff + 128],
                        vv4[:, i, h, :],
                        start=(i == 0), stop=(i == SB - 1))
                rec = rp.tile([128, 1], F32, tag="rec")
                nc.vector.reciprocal(rec[:], po[:, DK:V65])
                nc.vector.tensor_scalar_mul(
                    at3[:, sblk, h * DK:(h + 1) * DK],
                    po[:, 0:DK], rec[:])
                if with_atT:
                    nc.sync.dma_start_transpose(
                        atT3[:, :, sblk * 128:(sblk + 1) * 128],
                        at3[:, sblk, :])

            # ---- fill-work thunk factories (costs ~= full-speed PE ns) ----
            proj_ctr = [0]

            def qkp4(dblk, sc):
                """proj_qkT as 4 thunks of 2 contraction chunks (~426ns).
                Accumulators alternate the pq and pso banks (pso is idle
                until the first out-projections in P7, and projections end
                by P5), so consecutive sets never serialize through one
                bank's write-after-read on the DVE bias-add."""
                box = []
                npj = proj_ctr[0]
                proj_ctr[0] += 1
                ppool, ptag = (pq, "pq") if npj % 2 == 0 else (pso, "pso")

                def mk(j):
                    def t():
                        if j == 0:
                            box.append(ppool.tile([128, 512], F32, tag=ptag,
                                                  name="pqt"))
                        ps = box[0]
                        for c in (2 * j, 2 * j + 1):
                            nc.tensor.matmul(
                                ps[:],
                                wqkT3[:, dblk, c, :],
                                xt3[:, c, sc * 512:(sc + 1) * 512],
                                start=(c == 0), stop=(c == KC - 1))
                        if j == 3:
                            nc.vector.tensor_scalar_add(
                                qk3[:, dblk, sc * 512:(sc + 1) * 512],
                                ps[:], bqk[:, dblk:dblk + 1])
                    return (426, t)
                return [mk(j) for j in range(4)]

            def vph(i):
                """proj_v for s-block i as 2 thunks of 4 chunks (~427ns)."""
                box = []

                def mk(j):
                    def t():
                        if j == 0:
                            box.append(ppv.tile([128, 512], F32, tag="ppv", name="vpt"))
                        ps = box[0]
                        for c in range(4 * j, 4 * j + 4):
                            nc.tensor.matmul(
                                ps[:, 0:DQK],
                                xt3[:, c, i * 128:(i + 1) * 128],
                                wvT3[:, c, :],
                                start=(c == 0), stop=(c == KC - 1))
                        if j == 1:
                            nc.vector.tensor_add(
                                vv4[:, i, :, 0:DK],
                                ps[:, 0:DQK].rearrange(
                                    "p (h d) -> p h d", h=HPC),
                                bv128[:].rearrange("p (h d) -> p h d", h=HPC))
                    return (427, t)
                return [mk(j) for j in range(2)]

            def pvset(qc, h, exs_ref, with_atT=False, rotate=False):
                """8 PV thunks (one per q-block, ~433ns each). rotate:
                spread accumulators over ppv+pq+pso (only valid where the
                projection/out-proj banks are idle, i.e. pass P6) so
                back-to-back PV units don't serialize on ppv's two bufs."""
                cyc = [None, None, (pq, "pq"), (pso, "pso")]

                def mk(qb):
                    pool = cyc[qb % 4] if rotate else None
                    return (433, lambda: pv_one(qc, h, exs_ref(), qb,
                                                with_atT=with_atT,
                                                pool=pool))
                return [mk(qb) for qb in range(SBH)]

            op_ctr = [0]

            def oph(sblk):
                """out-projection for s-block as 2 thunks (~426ns each).
                PSUM alternates between the pso bank and the pq bank (idle
                after the projections finish) so consecutive halves don't
                serialize on one bank; the bias-add alternates DVE/Pool so
                neither engine paces the op stream."""
                box = []

                def mk(dmh):
                    def t():
                        if dmh == 0:
                            box.append(op_.tile([128, DIN], BF16, tag="ot",
                                                name="ott"))
                        ot = box[0]
                        n = op_ctr[0]
                        op_ctr[0] += 1
                        pool = pso if n % 2 == 0 else pq
                        ps = pool.tile([128, 512], F32,
                                       tag="pso" if n % 2 == 0 else "pq",
                                       name="opps")
                        for ch in range(2):
                            nc.tensor.matmul(
                                ps[:],
                                atT3[:, ch, sblk * 128:(sblk + 1) * 128],
                                woT3[:, ch, dmh * 512:(dmh + 1) * 512],
                                start=(ch == 0), stop=(ch == 1))
                        nc.vector.tensor_add(
                            ot[:, dmh * 512:(dmh + 1) * 512], ps[:],
                            bo128[:, dmh * 512:(dmh + 1) * 512])
                        if dmh == 1:
                            nc.sync.dma_start(
                                out_d.ap()[sblk * 128:(sblk + 1) * 128, :],
                                ot[:])
                    return (426, t)
                return [mk(0), mk(1)]

            # ---- global fill queue + drain ----
            from collections import deque
            work_q = deque()
            credit = [0.0]

            def drain(budget):
                avail = budget + credit[0]
                while work_q and avail >= 0.5 * work_q[0][0]:
                    c, fn = work_q.popleft()
                    fn()
                    avail -= c
                credit[0] = min(max(avail, 0.0), 100.0)

            def push(*thunk_lists):
                for tl in thunk_lists:
                    work_q.extend(tl)

            # ---- pass schedule ----
            # P1=(h0,qc0) P2=(h0,qc1) P3=(h1,qc0) P4=(h1,qc1)
            # P5=(h2,qc0) P6=(h3,qc0) P7=(h2,qc1) P8=(h3,qc1) in halves.
            # h0/h1 share q0/k0 projections; h2/h3 share q1/k1 — the hybrid
            # order spreads projection deadlines and keeps qc0 out-proj off
            # the tail.
            push(qkp4(2, 1), qkp4(2, 2), qkp4(2, 3), qkp4(0, 2), qkp4(0, 3))
            ex_p1 = scores_exp(0, 0, drain)
            for i in range(SB):
                push(vph(i))
            ex_p2 = scores_exp(1, 0, drain)
            push(pvset(0, 0, lambda: ex_p1))
            push(qkp4(1, 0), qkp4(1, 1))
            ex_p3 = scores_exp(0, 1, drain)
            push(qkp4(3, 0), qkp4(3, 1), qkp4(3, 2), qkp4(3, 3))
            push(pvset(1, 0, lambda: ex_p2))
            ex_p4 = scores_exp(1, 1, drain)
            push(qkp4(1, 2), qkp4(1, 3))
            push(pvset(0, 1, lambda: ex_p3))
            ex_p5 = scores_exp(0, 2, drain)
            push(pvset(1, 1, lambda: ex_p4, rotate=True))
            push(pvset(0, 2, lambda: ex_p5, rotate=True))
            ex_p6 = scores_exp(0, 3, drain)
            push(pvset(0, 3, lambda: ex_p6, with_atT=True))
            for k in range(SBH):
                push(oph(k))
            ex_p7 = scores_exp(1, 2, drain)
            push(pvset(1, 2, lambda: ex_p7))
            ex_p8a = scores_exp(1, 3, drain, half_n2=0)
            pvh_a = [t for _, t in pvset(1, 3, lambda: ex_p8a,
                                         with_atT=True)][:4]
            push([(433, pvh_a[0]), (433, pvh_a[1])],
                 oph(SBH), [(433, pvh_a[2])],
                 oph(SBH + 1), [(433, pvh_a[3])],
                 oph(SBH + 2), oph(SBH + 3))
            ex_p8b = scores_exp(1, 3, drain, half_n2=1)

            def warm1():
                # p-state keep-alive during the dependency-latency-bound
                # tail (pss is free once the last score slab is done)
                psw = pss.tile([128, QC], F32, tag="pss", name="pswt")
                nc.tensor.matmul(psw[:, 0:128], warm[:], warm[:],
                                 start=True, stop=True)

            def pv_tail(qb):
                # final-head PV + PE-transpose of the at row (the serial
                # DMA-transpose path costs ~1.8us per block in the tail)
                pv_one(1, 3, ex_p8b, qb, half=True)
                sblk = SBH + qb
                pst = pss.tile([128, QC], F32, tag="pss", name="pstt")
                pstb = pst.bitcast(BF16)
                for ch in range(2):
                    nc.tensor.matmul(
                        pstb[:, ch * 128:(ch + 1) * 128],
                        at3[:, sblk, ch * 128:(ch + 1) * 128],
                        ident[:], is_transpose=True,
                        start=(ch == 0), stop=(ch == 1),
                        skip_group_check=True)
                nc.vector.tensor_copy(
                    atT3[:, :, sblk * 128:(sblk + 1) * 128],
                    pstb[:, 0:256].rearrange("p (h j) -> p h j", h=2))


            def oph_tail(sblk, dmh, on_act, use_ppv=False):
                # tail out-proj half; on_act folds the bias in via a 1-deep
                # ones x b_out matmul and copies PSUM->SBUF on the (idle)
                # ACT engine so DVE and ACT each carry half the tail stream
                ot = ot_tail[sblk - SBH - 4]
                if use_ppv:
                    # ppv is idle once the last PV unit is done; a third
                    # bank removes the pso/pq write-after-read waits from
                    # the final op halves
                    ps = ppv.tile([128, 512], F32, tag="ppv", name="opts")
                else:
                    n = op_ctr[0]
                    op_ctr[0] += 1
                    pool = pso if n % 2 == 0 else pq
                    ps = pool.tile([128, 512], F32,
                                   tag="pso" if n % 2 == 0 else "pq",
                                   name="opts")
                if on_act:
                    nc.tensor.matmul(
                        ps[:], ones1[:], bo_bf[0:1,
                                               dmh * 512:
                                               (dmh + 1) * 512],
                        start=True, stop=False, skip_group_check=True)
                for ch in range(2):
                    nc.tensor.matmul(
                        ps[:],
                        atT3[:, ch, sblk * 128:(sblk + 1) * 128],
                        woT3[:, ch, dmh * 512:(dmh + 1) * 512],
                        start=(not on_act and ch == 0), stop=(ch == 1),
                        skip_group_check=True)
                if on_act:
                    nc.scalar.activation(
                        ot[:, dmh * 512:(dmh + 1) * 512], ps[:],
                        AF.Copy)
                else:
                    nc.vector.tensor_add(
                        ot[:, dmh * 512:(dmh + 1) * 512], ps[:],
                        bo128[:, dmh * 512:(dmh + 1) * 512])
                if dmh == 1:
                    nc.sync.dma_start(
                        out_d.ap()[sblk * 128:(sblk + 1) * 128, :], ot[:])

            # tail: drain leftovers, then final PV half (PE transpose) +
            # out-proj with ACT/DVE alternation; warm matmuls keep the PE
            # clock at full speed across dependency stalls
            ot_tail = [op_.tile([128, DIN], BF16, tag="ot",
                                name=f"ott{k}") for k in range(4)]
            pv_tail(4)
            pv_tail(5)
            drain(10**9)
            oph_tail(SBH + 4, 0, True)
            pv_tail(6)
            oph_tail(SBH + 4, 1, False)
            pv_tail(7)
            oph_tail(SBH + 5, 0, True)
            oph_tail(SBH + 5, 1, False)
            oph_tail(SBH + 6, 0, True, use_ppv=True)
            oph_tail(SBH + 6, 1, False)
            oph_tail(SBH + 7, 0, True, use_ppv=True)
            oph_tail(SBH + 7, 1, False)

    nc.compile()
    return nc


def shard_inputs(x, w_qkv, b_qkv, w_out, b_out, cfg: Cfg = FULL):
    """Build the 8 per-core input maps from full inputs (host-side layout
    marshaling: transpose/reshape/stack/dtype-cast, no arithmetic)."""
    DIN, DQK, KC, S = cfg.DIN, cfg.DQK, cfg.KC, cfg.S
    D = DIN
    bf16 = mybir.dt.np(mybir.dt.bfloat16)
    x = np.asarray(x, dtype=np.float32)
    w_qkv = np.asarray(w_qkv, dtype=np.float32)
    b_qkv = np.asarray(b_qkv, dtype=np.float32)
    w_out = np.asarray(w_out, dtype=np.float32)
    b_out = np.asarray(b_out, dtype=np.float32)
    zeros_bo = np.zeros((128, DIN), dtype=np.float32)
    bo128 = np.ascontiguousarray(
        np.broadcast_to(b_out.reshape(1, DIN), (128, DIN)))

    # x^T images per batch, as 2-s-block slabs:
    # row k2*128+p, col c*256+j = x[k2*256+j, c*128+p]
    xt_imgs = []
    for b in range(2):
        arr = x[b].astype(bf16).reshape(8, 256, KC, 128)  # (k2, j, c, p)
        xt_imgs.append(np.ascontiguousarray(
            arr.transpose(0, 3, 2, 1).reshape(8 * 128, S)))

    in_maps = []
    for c in range(cfg.N_CORES):
        b, hg = divmod(c, 4)
        sl = slice(hg * DQK, (hg + 1) * DQK)
        wq = w_qkv[0 * D:1 * D][sl]
        wk = w_qkv[1 * D:2 * D][sl]
        wv = w_qkv[2 * D:3 * D][sl]
        wo = w_out[:, sl]
        bq = b_qkv[0 * D:1 * D][sl]
        bk = b_qkv[1 * D:2 * D][sl]
        bqk_np = np.stack([bq[0:128], bq[128:256],
                           bk[0:128], bk[128:256]], axis=1)
        # w_qkT image [128, 4*1024]: col b*1024+c*128+j = W[b*128+j, c*128+p]
        wqk = np.concatenate([wq, wk], axis=0).astype(bf16)  # [512, 1024]
        wqkT = (wqk.reshape(4, 128, KC, 128)            # (blk, j, c, p)
                .transpose(3, 0, 2, 1).reshape(128, 4 * DIN))
        # w_vT image [128, 8*256]: col c*256+d = Wv[d, c*128+p]
        wvT = (wv.astype(bf16).reshape(DQK, KC, 128)    # (d, c, p)
               .transpose(2, 1, 0).reshape(128, KC * DQK))
        # w_oT image [128, 2*1024]: col ch*1024+dm = Wo[dm, ch*128+p]
        woT = (wo.astype(bf16).reshape(DIN, 2, 128)     # (dm, ch, p)
               .transpose(2, 1, 0).reshape(128, 2 * DIN))
        bv128 = np.broadcast_to(
            b_qkv[2 * D:3 * D][sl].reshape(1, DQK), (128, DQK))
        in_maps.append({
            "ident": np.eye(128, dtype=bf16),
            "xt": xt_imgs[b],
            "w_qkT": np.ascontiguousarray(wqkT),
            "w_vT": np.ascontiguousarray(wvT),
            "w_oT": np.ascontiguousarray(woT),
            "b_qk": np.ascontiguousarray(bqk_np),
            "b_v128": np.ascontiguousarray(bv128),
            "b_o128": bo128 if hg == 0 else zeros_bo,
        })
    return in_maps


def gather_output(results, cfg: Cfg = FULL):
    outs = []
    for b in range(2):
        acc = results[4 * b]["out_partial"].astype(np.float32)
        for c in range(4 * b + 1, 4 * b + 4):
            acc = acc + results[c]["out_partial"].astype(np.float32)
        outs.append(acc)
    return np.stack(outs, axis=0)


_NC_CACHE = {}


def _get_nc(cfg: Cfg = FULL):
    if cfg not in _NC_CACHE:
        _NC_CACHE[cfg] = build_nc(cfg)
    return _NC_CACHE[cfg]


def kernel(x, w_qkv, b_qkv, w_out, b_out):
    cfg = FULL
    nc = _get_nc(cfg)
    in_maps = shard_inputs(x, w_qkv, b_qkv, w_out, b_out, cfg)
    res = run_bass_kernel_spmd(nc, in_maps, core_ids=list(range(cfg.N_CORES)))
    return gather_output(res.results, cfg)


if __name__ == "__main__":
    rng = np.random.default_rng(0)
    D = FULL.DIN
    x = rng.standard_normal((2, FULL.S, D), dtype=np.float32)
    w_qkv = (rng.standard_normal((3 * D, D), dtype=np.float32) / np.sqrt(D))
    b_qkv = rng.standard_normal(3 * D, dtype=np.float32) * 0.02
    w_out = rng.standard_normal((D, D), dtype=np.float32) / np.sqrt(D)
    b_out = rng.standard_normal(D, dtype=np.float32) * 0.02
    out = kernel(x=x, w_qkv=w_qkv, b_qkv=b_qkv, w_out=w_out, b_out=b_out)
    print("out", out.shape, out.dtype, float(np.abs(out).mean()))

